# revision 35
# baseline (speedup 1.0000x reference)
"""Expert-parallel MoE SwiGLU kernel for one TRN2 chip (8 NeuronCores).

Problem: out[n] = sum_k w[n,k] * FFN_{idx[n,k]}(x[n]) with E=8 experts,
top-2 routing, H=1024, I=4096, N=2048 tokens.

Strategy: one expert per core. Tokens are routed (gathered) per expert on
the host, each core runs the three bf16 matmuls of its expert's SwiGLU FFN
(silu(x@w1) * (x@w3)) @ w2 over its token batch entirely transposed
(tokens along the PE moving/free dim), and the host scatter-adds the
returned per-expert outputs with the routing weights. Expert token counts
above the per-core capacity (PE moving-dim chunk of 512) spill to a small
host-side f32 pass so the device runs a single full-width chunk.

DMA schedule: the PE-critical startup transfers (x chunks + the first
w1/w3 tile, split into small pieces) ride alone on the two hardware DGE
rings (sync + scalar) so the first real matmul starts as early as
possible. Everything else (w13 tiles 1..31, then the 8 w2 tiles) streams
on the gpsimd software-DGE ring, serialized by WAW micro-DMA chaining so
the Tile scheduler cannot hoist the whole weight set into one initial
burst that starves the x load (which is what limited the previous
version to a ~20us PE start).
"""

import sys

for _p in ("/opt/trn_rl_repo", "/opt/pypackages"):
    if _p not in sys.path:
        sys.path.insert(0, _p)

import numpy as np
import ml_dtypes

import concourse.tile as tile
from concourse import bacc, mybir
from concourse.bass_utils import run_bass_kernel_spmd

P = 128
H = 1024
I = 4096
KH = H // P    # 8 contraction subtiles for the first matmuls
II = I // P    # 32 intermediate subtiles / contraction subtiles for w2
CAP = 448      # per-core token capacity (capacity factor 0.875; overflow is
               # computed exactly in f32 on the host, so accuracy is
               # unaffected -- the device just runs a shorter PE chunk)
N_WARM = 13    # PE warmup matmuls sized to bridge until the x DMA lands
               # (~12-13us) so the clock is fully ramped and the real stream
               # never idles (an idle gap resets the HAM ramp and costs ~7us
               # of half-speed matmuls)

BF16 = mybir.dt.bfloat16
F32 = mybir.dt.float32


def _build(C):
    """One-expert SwiGLU FFN over C tokens (C <= 512), transposed layout.

    DRAM inputs (per core):
      xg   [P, KH, C]       bf16  x^T: [hp, kh, c] = x[tok c, kh*P+hp]
      w13t [II, P, 2, KH, P] bf16 [ii, hp, 0, kh, m] = w1[kh*P+hp, ii*P+m]
                                  [ii, hp, 1, kh, m] = w3[kh*P+hp, ii*P+m]
      w2t  [KH, P, II, P]   bf16  [hh, ip, ik, m] = w2[ik*P+ip, hh*P+m]
    Output:
      yt   [KH, P, C]       f32   y^T tiled by output subtile
    """
    assert C <= 512
    nc = bacc.Bacc("TRN2", target_bir_lowering=False, debug=False)
    xg = nc.dram_tensor("xg", [P, KH, C], BF16, kind="ExternalInput")
    w13t = nc.dram_tensor("w13t", [II, P, 2, KH, P], BF16, kind="ExternalInput")
    w2t = nc.dram_tensor("w2t", [KH, P, II, P], BF16, kind="ExternalInput")
    yt = nc.dram_tensor("yt", [KH, P, C], F32, kind="ExternalOutput")

    with tile.TileContext(nc) as tc:
        with (
            tc.tile_pool(name="xp", bufs=2) as xp,
            tc.tile_pool(name="pp", bufs=1) as pp,
            tc.tile_pool(name="wp", bufs=4) as wp,
            tc.tile_pool(name="w2p", bufs=3) as w2p,
            tc.tile_pool(name="gp", bufs=4) as gp,
            tc.tile_pool(name="yp", bufs=3) as yp,
            tc.tile_pool(name="warm", bufs=1) as warm,
            tc.tile_pool(name="psA", bufs=3, space="PSUM") as psA,
            tc.tile_pool(name="psB", bufs=2, space="PSUM") as psB,
        ):
            # PE warmup: ramp the tensor engine to high-activity clock while
            # the input DMAs are in flight. Reads a zeroed tile, result is
            # never consumed. Short (N_WARM small): x now arrives ~10us
            # earlier than before, and warmups queued ahead of real matmuls
            # would delay them.
            wtile = warm.tile([P, 512], BF16)
            nc.vector.memset(wtile[:], 0.0)
            wps = psB.tile([P, 512], F32, tag="py")
            for i in range(N_WARM):
                nc.tensor.matmul(
                    wps, wtile[:, :P], wtile[:], start=(i == 0),
                    stop=(i == N_WARM - 1),
                )

            # Startup-critical loads. DMA ring throughput here is descriptor-
            # rate-bound, so each transfer uses the largest per-partition
            # contiguous runs the layouts allow (4KB -> 128 descriptors per
            # 512KB). The gpsimd SW ring moves these ~3x faster than a HW
            # ring, so the whole w13 stream (w13[0] included) lives there,
            # while x rides the two HW rings -- nothing competes with
            # anything on any ring:
            #   sync ring (HW):   x kh0-3, later w2[0..2] + yt outputs
            #   scalar ring (HW): x kh4-7 (then only silus/copies)
            #   gpsimd ring (SW): w13[0..31] paced by the wp pool (bufs=3:
            #                     at most 3 hoisted at t=0, so even the
            #                     worst ring order lands w13[0] by ~x time)
            xc = []
            for q in range(2):
                xt = xp.tile([P, 4, C], BF16, tag="x")
                xc.append(xt)
            nc.sync.dma_start(xc[0][:], xg[:, 0:4, :])
            w13sb0 = wp.tile([P, 2, KH, P], BF16, tag="w13")
            nc.gpsimd.dma_start(w13sb0[:], w13t[0])
            nc.scalar.dma_start(xc[1][:], xg[:, 4:8, :])

            psb = pp.tile([P, II, C], BF16)

            def xmov(kh):
                return xc[kh // 4][:, kh % 4, :]

            def head2(t):
                # first 2 elements of partition 0, for fake-dep micro-DMAs
                if len(t.shape) == 4:
                    return t[0:1, 0:1, 0:1, 0:2]
                if len(t.shape) == 3:
                    return t[0:1, 0:1, 0:2]
                return t[0:1, 0:2]

            # Phase A: h1 = silu(x@w1), h3 = x@w3, p = h1*h3 (all transposed).
            # w13[1..4] would otherwise be hoisted into the startup burst and
            # starve the x load of DMA bandwidth; each gets a single parallel
            # fake RAW dep (2-element sbuf->sbuf DMA) on an x chunk, so they
            # start only once x is (mostly) in. No chaining -- the stream
            # still pipelines. From w13[5] on, the wp pool depth paces the
            # prefetch off PE consumption (buffer reuse WAR deps).
            for ii in range(II):
                if ii == 0:
                    w13sb = w13sb0
                else:
                    w13sb = wp.tile([P, 2, KH, P], BF16, tag="w13")
                    nc.gpsimd.dma_start(w13sb[:], w13t[ii])
                pg = psA.tile([P, C], F32, tag="pg")
                pu = psA.tile([P, C], F32, tag="pu")
                for kh in range(KH):
                    nc.tensor.matmul(
                        pg,
                        w13sb[:, 0, kh, :],
                        xmov(kh),
                        start=(kh == 0),
                        stop=(kh == KH - 1),
                    )
                for kh in range(KH):
                    nc.tensor.matmul(
                        pu,
                        w13sb[:, 1, kh, :],
                        xmov(kh),
                        start=(kh == 0),
                        stop=(kh == KH - 1),
                    )
                gs = gp.tile([P, C], BF16, tag="g")
                nc.scalar.activation(gs, pg, mybir.ActivationFunctionType.Silu)
                nc.vector.tensor_tensor(
                    psb[:, ii, :], gs, pu, mybir.AluOpType.mult
                )

            # Phase B: y = p @ w2 (transposed: yT = w2T-contraction over I).
            # The first three w2 tiles have free pool buffers at t=0 and
            # would be hoisted into the startup burst; a tiny fake RAW dep on
            # a mid-phase-A w13 tile delays each until the startup DMAs are
            # long done. w2[3..7] are paced by w2p pool reuse during phase B.
            # The last hh is split column-wise so earlier pieces' copy+DMA
            # overlap the later pieces' matmuls (shorter kernel tail).
            # w2[0..2] ride the sync HW ring (idle during phase A) so they do
            # not delay the w13 stream on the gpsimd ring. Each is gated on a
            # mid-phase-A psb slice: psb is a single never-reused tile, so
            # the gate reads cannot block any pool rotation, and the releases
            # sit long before the phase-B output DMAs queue on sync.
            # w2[3..7] ride gpsimd, which is idle during phase B.
            w2sb_all = []
            for hh in range(KH):
                w2sb = w2p.tile([P, II, P], BF16, tag="w2")
                if hh < 3:
                    src = 14 + 2 * hh
                    nc.sync.dma_start(
                        head2(w2sb), psb[0:1, src : src + 1, 0:2]
                    )
                    nc.sync.dma_start(w2sb[:], w2t[hh])
                else:
                    nc.gpsimd.dma_start(w2sb[:], w2t[hh])
                w2sb_all.append(w2sb)

            for hh in range(KH):
                w2sb = w2sb_all[hh]
                if hh < KH - 1:
                    pieces = [(0, C)]
                else:
                    pieces = [(0, C // 2), (C // 2, C // 4), (3 * C // 4, C // 4)]
                for c0, cc in pieces:
                    py = psB.tile([P, cc], F32, tag="py")
                    for ik in range(II):
                        nc.tensor.matmul(
                            py,
                            w2sb[:, ik, :],
                            psb[:, ik, c0 : c0 + cc],
                            start=(ik == 0),
                            stop=(ik == II - 1),
                        )
                    ysb = yp.tile([P, cc], F32, tag="y")
                    nc.scalar.copy(ysb, py)
                    nc.sync.dma_start(yt[hh, :, c0 : c0 + cc], ysb[:])

    nc.compile()
    return nc


_PROGRAM_CACHE = {}


def _host_swiglu(x, w1e, w2e, w3e):
    g = x @ w1e
    u = x @ w3e
    g = g / (1.0 + np.exp(-g))
    return (g * u) @ w2e


def kernel(x, expert_indices, expert_weights, w1, w2, w3):
    x = np.asarray(x, dtype=np.float32)
    idx = np.asarray(expert_indices)
    wts = np.asarray(expert_weights, dtype=np.float32)
    w1 = np.asarray(w1, dtype=np.float32)
    w2 = np.asarray(w2, dtype=np.float32)
    w3 = np.asarray(w3, dtype=np.float32)
    N = x.shape[0]
    E = w1.shape[0]
    bf16 = ml_dtypes.bfloat16

    # host-side routing: token list (with multiplicity) per expert; tokens
    # beyond CAP spill to the host f32 path (tiny tail, keeps device at one
    # full-width PE chunk)
    toks, tokw, spill_toks, spill_w = [], [], [], []
    for e in range(E):
        rows, cols = np.nonzero(idx == e)
        w_e = wts[rows, cols]
        toks.append(rows[:CAP])
        tokw.append(w_e[:CAP])
        spill_toks.append(rows[CAP:])
        spill_w.append(w_e[CAP:])
    C = max(16, max(len(t) for t in toks))
    C = ((C + 15) // 16) * 16

    if C not in _PROGRAM_CACHE:
        _PROGRAM_CACHE[C] = _build(C)
    nc = _PROGRAM_CACHE[C]

    in_maps = []
    for e in range(E):
        xt = np.zeros((C, H), dtype=np.float32)
        if len(toks[e]):
            xt[: len(toks[e])] = x[toks[e]]
        # [C, H] -> [hp, kh, c]
        xge = xt.T.reshape(KH, P, C).transpose(1, 0, 2)
        # w1/w3 [H, I] -> [ii, hp, {w1,w3}, kh, m]
        w13 = np.stack(
            [
                w1[e].reshape(KH, P, II, P).transpose(2, 1, 0, 3),
                w3[e].reshape(KH, P, II, P).transpose(2, 1, 0, 3),
            ],
            axis=2,
        )  # [II, P, 2, KH, P]
        in_maps.append(
            {
                "xg": np.ascontiguousarray(xge.astype(bf16)),
                "w13t": np.ascontiguousarray(w13.astype(bf16)),
                "w2t": np.ascontiguousarray(
                    w2[e].reshape(II, P, KH, P).transpose(2, 1, 0, 3).astype(bf16)
                ),
            }
        )

    res = run_bass_kernel_spmd(nc, in_maps, core_ids=list(range(E)))

    out = np.zeros((N, H), dtype=np.float32)
    for e in range(E):
        cnt = len(toks[e])
        if cnt:
            y = res.results[e]["yt"].reshape(H, C).T[:cnt]
            np.add.at(out, toks[e], y * tokw[e][:, None])
        if len(spill_toks[e]):
            ys = _host_swiglu(x[spill_toks[e]], w1[e], w2[e], w3[e])
            np.add.at(out, spill_toks[e], ys * spill_w[e][:, None])
    return out


# revision 38
# speedup vs baseline: 1.1976x; 1.1976x over previous
"""Expert-parallel MoE SwiGLU kernel for one TRN2 chip (8 NeuronCores).

Problem: out[n] = sum_k w[n,k] * FFN_{idx[n,k]}(x[n]) with E=8 experts,
top-2 routing, H=1024, I=4096, N=2048 tokens.

Strategy: one expert per core. Tokens are routed (gathered) per expert on
the host, each core runs the three bf16 matmuls of its expert's SwiGLU FFN
(silu(x@w1) * (x@w3)) @ w2 over its token batch entirely transposed
(tokens along the PE moving/free dim), and the host scatter-adds the
returned per-expert outputs with the routing weights. Expert token counts
above the per-core capacity (PE moving-dim chunk of 512) spill to a small
host-side f32 pass so the device runs a single full-width chunk.

DMA schedule: the PE-critical startup transfers (x chunks + the first
w1/w3 tile, split into small pieces) ride alone on the two hardware DGE
rings (sync + scalar) so the first real matmul starts as early as
possible. Everything else (w13 tiles 1..31, then the 8 w2 tiles) streams
on the gpsimd software-DGE ring, serialized by WAW micro-DMA chaining so
the Tile scheduler cannot hoist the whole weight set into one initial
burst that starves the x load (which is what limited the previous
version to a ~20us PE start).
"""

import sys

for _p in ("/opt/trn_rl_repo", "/opt/pypackages"):
    if _p not in sys.path:
        sys.path.insert(0, _p)

import numpy as np
import ml_dtypes

import concourse.tile as tile
from concourse import bacc, mybir
from concourse.bass_utils import run_bass_kernel_spmd

P = 128
H = 1024
I = 4096
KH = H // P    # 8 contraction subtiles for the first matmuls
II = I // P    # 32 intermediate subtiles / contraction subtiles for w2
CAP = 448      # per-core token capacity (capacity factor 0.875; overflow is
               # computed exactly in f32 on the host, so accuracy is
               # unaffected -- the device just runs a shorter PE chunk)
N_WARM = 12    # PE warmup matmuls sized to bridge until the x DMA lands
               # (~12-13us) so the clock is fully ramped and the real stream
               # never idles (an idle gap resets the HAM ramp and costs ~7us
               # of half-speed matmuls)

BF16 = mybir.dt.bfloat16
F32 = mybir.dt.float32


def _build(C):
    """One-expert SwiGLU FFN over C tokens (C <= 512), transposed layout.

    DRAM inputs (per core):
      xg   [P, KH, C]       bf16  x^T: [hp, kh, c] = x[tok c, kh*P+hp]
      w13t [II, P, 2, KH, P] bf16 [ii, hp, 0, kh, m] = w1[kh*P+hp, ii*P+m]
                                  [ii, hp, 1, kh, m] = w3[kh*P+hp, ii*P+m]
      w2t  [KH, P, II, P]   bf16  [hh, ip, ik, m] = w2[ik*P+ip, hh*P+m]
    Output:
      yt   [KH, P, C]       f32   y^T tiled by output subtile
    """
    assert C <= 512
    nc = bacc.Bacc("TRN2", target_bir_lowering=False, debug=False)
    xg = nc.dram_tensor("xg", [P, KH, C], BF16, kind="ExternalInput")
    w13t = nc.dram_tensor("w13t", [II, P, 2, KH, P], BF16, kind="ExternalInput")
    w2t = nc.dram_tensor("w2t", [KH, P, II, P], BF16, kind="ExternalInput")
    yt = nc.dram_tensor("yt", [KH, P, C], F32, kind="ExternalOutput")

    with tile.TileContext(nc) as tc:
        with (
            tc.tile_pool(name="xp", bufs=2) as xp,
            tc.tile_pool(name="pp", bufs=1) as pp,
            tc.tile_pool(name="wp", bufs=4) as wp,
            tc.tile_pool(name="w2p", bufs=3) as w2p,
            tc.tile_pool(name="gp", bufs=4) as gp,
            tc.tile_pool(name="yp", bufs=3) as yp,
            tc.tile_pool(name="warm", bufs=1) as warm,
            tc.tile_pool(name="psA", bufs=3, space="PSUM") as psA,
            tc.tile_pool(name="psB", bufs=2, space="PSUM") as psB,
        ):
            # PE warmup: ramp the tensor engine to high-activity clock while
            # the input DMAs are in flight. Reads a zeroed tile, result is
            # never consumed. Short (N_WARM small): x now arrives ~10us
            # earlier than before, and warmups queued ahead of real matmuls
            # would delay them.
            wtile = warm.tile([P, 512], BF16)
            nc.vector.memset(wtile[:], 0.0)
            wps = psB.tile([P, 512], F32, tag="py")
            for i in range(N_WARM):
                nc.tensor.matmul(
                    wps, wtile[:, :P], wtile[:], start=(i == 0),
                    stop=(i == N_WARM - 1),
                )

            # Startup-critical loads. DMA ring throughput here is descriptor-
            # rate-bound, so each transfer uses the largest per-partition
            # contiguous runs the layouts allow (4KB -> 128 descriptors per
            # 512KB). The gpsimd SW ring moves these ~3x faster than a HW
            # ring, so the whole w13 stream (w13[0] included) lives there,
            # while x rides the two HW rings -- nothing competes with
            # anything on any ring:
            #   sync ring (HW):   x kh0-3, later w2[0..2] + yt outputs
            #   scalar ring (HW): x kh4-7 (then only silus/copies)
            #   gpsimd ring (SW): w13[0..31] paced by the wp pool (bufs=3:
            #                     at most 3 hoisted at t=0, so even the
            #                     worst ring order lands w13[0] by ~x time)
            xc = []
            for q in range(2):
                xt = xp.tile([P, 4, C], BF16, tag="x")
                xc.append(xt)
            nc.sync.dma_start(xc[0][:], xg[:, 0:4, :])
            w13sb0 = wp.tile([P, 2, KH, P], BF16, tag="w13")
            nc.gpsimd.dma_start(w13sb0[:], w13t[0])
            nc.scalar.dma_start(xc[1][:], xg[:, 4:8, :])

            psb = pp.tile([P, II, C], BF16)

            def xmov(kh):
                return xc[kh // 4][:, kh % 4, :]

            def head2(t):
                # first 2 elements of partition 0, for fake-dep micro-DMAs
                if len(t.shape) == 4:
                    return t[0:1, 0:1, 0:1, 0:2]
                if len(t.shape) == 3:
                    return t[0:1, 0:1, 0:2]
                return t[0:1, 0:2]

            # Phase A: h1 = silu(x@w1), h3 = x@w3, p = h1*h3 (all transposed).
            # w13[1..4] would otherwise be hoisted into the startup burst and
            # starve the x load of DMA bandwidth; each gets a single parallel
            # fake RAW dep (2-element sbuf->sbuf DMA) on an x chunk, so they
            # start only once x is (mostly) in. No chaining -- the stream
            # still pipelines. From w13[5] on, the wp pool depth paces the
            # prefetch off PE consumption (buffer reuse WAR deps).
            for ii in range(II):
                if ii == 0:
                    w13sb = w13sb0
                else:
                    w13sb = wp.tile([P, 2, KH, P], BF16, tag="w13")
                    if ii == 3:
                        # bufs=4 would hoist a 4th tile into the startup
                        # burst and delay x; gating it on x kh0-3 keeps the
                        # burst at 3 tiles while the deeper pool cushion
                        # absorbs the channel-steal of the w2 transfers
                        # mid-phase-A
                        nc.gpsimd.dma_start(head2(w13sb), head2(xc[0]))
                    nc.gpsimd.dma_start(w13sb[:], w13t[ii])
                pg = psA.tile([P, C], F32, tag="pg")
                pu = psA.tile([P, C], F32, tag="pu")
                for kh in range(KH):
                    nc.tensor.matmul(
                        pg,
                        w13sb[:, 0, kh, :],
                        xmov(kh),
                        start=(kh == 0),
                        stop=(kh == KH - 1),
                    )
                for kh in range(KH):
                    nc.tensor.matmul(
                        pu,
                        w13sb[:, 1, kh, :],
                        xmov(kh),
                        start=(kh == 0),
                        stop=(kh == KH - 1),
                    )
                gs = gp.tile([P, C], BF16, tag="g")
                nc.scalar.activation(gs, pg, mybir.ActivationFunctionType.Silu)
                nc.vector.tensor_tensor(
                    psb[:, ii, :], gs, pu, mybir.AluOpType.mult
                )

            # Phase B: y = p @ w2 (transposed: yT = w2T-contraction over I).
            # The first three w2 tiles have free pool buffers at t=0 and
            # would be hoisted into the startup burst; a tiny fake RAW dep on
            # a mid-phase-A w13 tile delays each until the startup DMAs are
            # long done. w2[3..7] are paced by w2p pool reuse during phase B.
            # The last hh is split column-wise so earlier pieces' copy+DMA
            # overlap the later pieces' matmuls (shorter kernel tail).
            # w2[0..2] ride the sync HW ring (idle during phase A) so they do
            # not delay the w13 stream on the gpsimd ring. Each is gated on a
            # mid-phase-A psb slice: psb is a single never-reused tile, so
            # the gate reads cannot block any pool rotation, and the releases
            # sit long before the phase-B output DMAs queue on sync.
            # w2[3..7] ride gpsimd, which is idle during phase B.
            w2sb_all = []
            for hh in range(KH):
                w2sb = w2p.tile([P, II, P], BF16, tag="w2")
                if hh < 3:
                    src = 14 + 2 * hh
                    nc.sync.dma_start(
                        head2(w2sb), psb[0:1, src : src + 1, 0:2]
                    )
                    nc.sync.dma_start(w2sb[:], w2t[hh])
                else:
                    nc.gpsimd.dma_start(w2sb[:], w2t[hh])
                w2sb_all.append(w2sb)

            for hh in range(KH):
                w2sb = w2sb_all[hh]
                if hh < KH - 1:
                    pieces = [(0, C)]
                else:
                    pieces = [(0, C // 2), (C // 2, C // 4), (3 * C // 4, C // 4)]
                for c0, cc in pieces:
                    py = psB.tile([P, cc], F32, tag="py")
                    for ik in range(II):
                        nc.tensor.matmul(
                            py,
                            w2sb[:, ik, :],
                            psb[:, ik, c0 : c0 + cc],
                            start=(ik == 0),
                            stop=(ik == II - 1),
                        )
                    ysb = yp.tile([P, cc], F32, tag="y")
                    nc.scalar.copy(ysb, py)
                    nc.sync.dma_start(yt[hh, :, c0 : c0 + cc], ysb[:])

    nc.compile()
    return nc


_PROGRAM_CACHE = {}


def _host_swiglu(x, w1e, w2e, w3e):
    g = x @ w1e
    u = x @ w3e
    g = g / (1.0 + np.exp(-g))
    return (g * u) @ w2e


def kernel(x, expert_indices, expert_weights, w1, w2, w3):
    x = np.asarray(x, dtype=np.float32)
    idx = np.asarray(expert_indices)
    wts = np.asarray(expert_weights, dtype=np.float32)
    w1 = np.asarray(w1, dtype=np.float32)
    w2 = np.asarray(w2, dtype=np.float32)
    w3 = np.asarray(w3, dtype=np.float32)
    N = x.shape[0]
    E = w1.shape[0]
    bf16 = ml_dtypes.bfloat16

    # host-side routing: token list (with multiplicity) per expert; tokens
    # beyond CAP spill to the host f32 path (tiny tail, keeps device at one
    # full-width PE chunk)
    toks, tokw, spill_toks, spill_w = [], [], [], []
    for e in range(E):
        rows, cols = np.nonzero(idx == e)
        w_e = wts[rows, cols]
        toks.append(rows[:CAP])
        tokw.append(w_e[:CAP])
        spill_toks.append(rows[CAP:])
        spill_w.append(w_e[CAP:])
    C = max(16, max(len(t) for t in toks))
    C = ((C + 15) // 16) * 16

    if C not in _PROGRAM_CACHE:
        _PROGRAM_CACHE[C] = _build(C)
    nc = _PROGRAM_CACHE[C]

    in_maps = []
    for e in range(E):
        xt = np.zeros((C, H), dtype=np.float32)
        if len(toks[e]):
            xt[: len(toks[e])] = x[toks[e]]
        # [C, H] -> [hp, kh, c]
        xge = xt.T.reshape(KH, P, C).transpose(1, 0, 2)
        # w1/w3 [H, I] -> [ii, hp, {w1,w3}, kh, m]
        w13 = np.stack(
            [
                w1[e].reshape(KH, P, II, P).transpose(2, 1, 0, 3),
                w3[e].reshape(KH, P, II, P).transpose(2, 1, 0, 3),
            ],
            axis=2,
        )  # [II, P, 2, KH, P]
        in_maps.append(
            {
                "xg": np.ascontiguousarray(xge.astype(bf16)),
                "w13t": np.ascontiguousarray(w13.astype(bf16)),
                "w2t": np.ascontiguousarray(
                    w2[e].reshape(II, P, KH, P).transpose(2, 1, 0, 3).astype(bf16)
                ),
            }
        )

    res = run_bass_kernel_spmd(nc, in_maps, core_ids=list(range(E)))

    out = np.zeros((N, H), dtype=np.float32)
    for e in range(E):
        cnt = len(toks[e])
        if cnt:
            y = res.results[e]["yt"].reshape(H, C).T[:cnt]
            np.add.at(out, toks[e], y * tokw[e][:, None])
        if len(spill_toks[e]):
            ys = _host_swiglu(x[spill_toks[e]], w1[e], w2[e], w3[e])
            np.add.at(out, spill_toks[e], ys * spill_w[e][:, None])
    return out
